# revision 5
# baseline (speedup 1.0000x reference)
"""MDTA (Restormer transposed channel attention) Trainium2 kernel.

Data-parallel over batch: 8 batch elements -> 8 NeuronCores, one each.

Per-core algorithm (all matmuls bf16 inputs, fp32 PSUM accumulation):
  - x is uploaded as bf16 and DMA'd into a vertically-shifted padded
    "x3 stack" resident in SBUF: logical row (ty*48+c) at free pos q holds
    x_pad[c, q + (ty-1)*258] over a 258x258 zero-padded image layout.
    Rows 0..127 live in tile A; the 16 leftover rows live in tile B as four
    image-quarters at partition bases 0/32/64/96 (32-aligned for the PE).
    Vertical conv taps come from partition placement, horizontal taps from
    free-dim AP offsets, so the 3x3 depthwise conv fuses into dense matmuls.
  - Phase 1: fused (1x1 conv + depthwise 3x3) for q,k: 6 matmuls per
    512-pixel chunk -> [96, 512] PSUM.  Chunks are transposed on the PE and
    accumulated into Gram matrices holding all per-head q.k^T blocks plus
    the squared L2 norms on the diagonals.
  - Attention: logits scaled by 1/max(||q||,eps), 1/max(||k||,eps) and
    temperature, masked block-diagonal softmax over 6-wide head blocks.
  - P2 = proj_w @ attn, then per-tap phase-2 weights C_tap^T = V_ext^T @
    P2^T are built on device (V_ext bakes the v-path 1x1 conv, depthwise
    weights and stack placement).
  - Phase 2: final output = sum over 3 dx of C^T stacks applied to the
    same x3 stack; 6 matmuls per chunk -> [48, 512] -> DMA out.
"""

import functools
import sys

if "/opt/trn_rl_repo" not in sys.path:
    sys.path.insert(0, "/opt/trn_rl_repo")

import ml_dtypes
import numpy as np

import concourse.bass as bass
import concourse.tile as tile
from concourse import bacc, mybir
from concourse import bass_utils

BF16 = ml_dtypes.bfloat16
F32 = np.float32

B, C, H, W = 8, 48, 256, 256
HEADS, HD = 8, 6
PW = W + 2                 # padded row width
PF = PW * (H + 2)          # padded flat image size
X3F = PF + 4               # x3-A buffer free size (pad for AP rearrange)
QROWS = 65                 # q-rows stored per B quarter
QF = QROWS * PW + 6        # B quarter free size
NCH = (H * W) // 512       # 128 chunks of 512 pixels (2 image rows)
EPS = 1e-12

bf = mybir.dt.bfloat16
f32 = mybir.dt.float32


def _winA(t, ci, dx):
    """A-tile rhs window: 512 output pixels of chunk ci at horiz tap dx."""
    off = (2 * ci + 1) * PW + dx
    return t[:, off:off + 2 * PW].rearrange("p (r w) -> p r w", w=PW)[:, :, 0:W]


def _winB(t, ci, dx):
    """B-tile rhs window (quarter k = ci//32, partitions 32k..32k+16)."""
    k = ci // 32
    off = (2 * (ci - 32 * k) + 1) * PW + dx
    return t[32 * k:32 * k + 16, off:off + 2 * PW].rearrange(
        "p (r w) -> p r w", w=PW)[:, :, 0:W]


def _tp(ci):
    return (96, 0) if ci // 32 == 3 else None


@functools.cache
def _build():
    nc = bacc.Bacc("TRN2", target_bir_lowering=False, debug=False)

    xb = nc.dram_tensor("xb", [C, H * W], bf, kind="ExternalInput").ap()
    wA_d = nc.dram_tensor("wA", [128, 3 * 96], bf, kind="ExternalInput").ap()
    wB_d = nc.dram_tensor("wB4", [128, 3 * 96], bf, kind="ExternalInput").ap()
    vA_d = nc.dram_tensor("vA", [48, 9 * 128], bf, kind="ExternalInput").ap()
    vB_d = nc.dram_tensor("vB4", [48, 3 * 128], bf, kind="ExternalInput").ap()
    projT_d = nc.dram_tensor("projT", [48, 48], bf, kind="ExternalInput").ap()
    id96_d = nc.dram_tensor("id96", [96, 96], bf, kind="ExternalInput").ap()
    eye48_d = nc.dram_tensor("eye48", [48, 48], f32, kind="ExternalInput").ap()
    mask_d = nc.dram_tensor("maskbd", [48, 48], f32, kind="ExternalInput").ap()
    temp_d = nc.dram_tensor("temppc", [48, 1], f32, kind="ExternalInput").ap()
    y = nc.dram_tensor("y", [C, H * W], f32, kind="ExternalOutput").ap()

    with tile.TileContext(nc) as tc:
        with (
            tc.tile_pool(name="const", bufs=1) as cpool,
            tc.tile_pool(name="x3", bufs=1) as x3pool,
            tc.tile_pool(name="work", bufs=3) as wpool,
            tc.tile_pool(name="small", bufs=1) as spool,
        ):
            # ---- constants to SBUF ----
            wA = cpool.tile([128, 3 * 96], bf)
            wB4 = cpool.tile([128, 3 * 96], bf)
            vA = cpool.tile([48, 9 * 128], bf)
            vB4 = cpool.tile([48, 3 * 128], bf)
            projT = cpool.tile([48, 48], bf)
            id96 = cpool.tile([96, 96], bf)
            eye48 = cpool.tile([48, 48], f32)
            maskbd = cpool.tile([48, 48], f32)
            temppc = cpool.tile([48, 1], f32)
            for dst, src in [(wA, wA_d), (wB4, wB_d), (vA, vA_d), (vB4, vB_d),
                             (projT, projT_d), (id96, id96_d),
                             (eye48, eye48_d), (maskbd, mask_d),
                             (temppc, temp_d)]:
                nc.sync.dma_start(dst[:], src[:])

            # ---- x3 stack: zero then DMA interiors ----
            x3A = x3pool.tile([128, X3F], bf)
            x3B = x3pool.tile([128, QF], bf)
            hf = X3F // 2
            nc.vector.memset(x3A[0:64, 0:hf], 0.0)
            nc.vector.memset(x3A[0:64, hf:X3F], 0.0)
            nc.gpsimd.memset(x3A[64:128, 0:hf], 0.0)
            nc.gpsimd.memset(x3A[64:128, hf:X3F], 0.0)
            nc.gpsimd.memset(x3B[:], 0.0)

            xbv = xb.rearrange("c (r w) -> c r w", w=W)

            def a_dst(t, p0, p1, o0, nrows):
                return t[p0:p1, o0:o0 + nrows * PW].rearrange(
                    "p (r w) -> p r w", w=PW)[:, :, 0:W]

            for ty in range(2):
                o0 = (2 - ty) * PW + 1
                nc.sync.dma_start(a_dst(x3A, ty * 48, ty * 48 + 48, o0, H),
                                  xbv)
            # ty=2: channels 0-31 in A (q-row = y), channels 32-47 quartered
            nc.sync.dma_start(a_dst(x3A, 96, 128, 1, H), xbv[0:32])
            for k in range(4):
                y0 = 64 * k
                nrows = min(QROWS, H - y0)
                nc.sync.dma_start(
                    a_dst(x3B, 32 * k, 32 * k + 16, 1, nrows),
                    xbv[32:48, y0:y0 + nrows])

            # ---- phase 1: qk fused conv + Gram accumulation ----
            with (
                tc.tile_pool(name="psG", bufs=1, space="PSUM") as gpool,
                tc.tile_pool(name="psqk", bufs=2, space="PSUM") as qkpool,
                tc.tile_pool(name="pst", bufs=2, space="PSUM") as tpool,
            ):
                G1 = gpool.tile([48, 96], f32)   # rows=k(e): [k.q | k.k]
                G2 = gpool.tile([48, 48], f32)   # q.q (diag used)
                for ci in range(NCH):
                    qk_ps = qkpool.tile([96, 512], f32)
                    for dx in range(3):
                        nc.tensor.matmul(
                            qk_ps[:], wA[:, dx * 96:(dx + 1) * 96],
                            _winA(x3A, ci, dx), start=(dx == 0), stop=False)
                        nc.tensor.matmul(
                            qk_ps[:],
                            wB4[32 * (ci // 32):32 * (ci // 32) + 16,
                                dx * 96:(dx + 1) * 96],
                            _winB(x3B, ci, dx), start=False, stop=(dx == 2),
                            tile_position=_tp(ci))
                    qk_sb = wpool.tile([96, 512], bf, tag="qksb")
                    nc.scalar.copy(qk_sb[:], qk_ps[:])
                    ps_t = tpool.tile([128, 384], bf)
                    for j in range(4):
                        nc.tensor.transpose(
                            ps_t[:, j * 96:(j + 1) * 96],
                            qk_sb[:, j * 128:(j + 1) * 128], id96[:])
                    qkT = wpool.tile([128, 384], bf, tag="qkT")
                    nc.vector.tensor_copy(qkT[:], ps_t[:])
                    for j in range(4):
                        first = (ci == 0 and j == 0)
                        last = (ci == NCH - 1 and j == 3)
                        nc.tensor.matmul(
                            G1[:], qkT[:, j * 96 + 48:j * 96 + 96],
                            qkT[:, j * 96:(j + 1) * 96],
                            start=first, stop=last)
                        nc.tensor.matmul(
                            G2[:], qkT[:, j * 96:j * 96 + 48],
                            qkT[:, j * 96:j * 96 + 48],
                            start=first, stop=last)

                Gs1 = spool.tile([48, 96], f32)
                Gs2 = spool.tile([48, 48], f32)
                nc.vector.tensor_copy(Gs1[:], G1[:])
                nc.vector.tensor_copy(Gs2[:], G2[:])

            # ---- attention (tiny, partitions 0-47) ----
            with tc.tile_pool(name="psS", bufs=1, space="PSUM") as sppool:
                gd = spool.tile([48, 48], f32)
                ssqk = spool.tile([48, 1], f32)
                ssqq = spool.tile([48, 1], f32)
                nk = spool.tile([48, 1], f32)
                nq = spool.tile([48, 1], f32)
                invk = spool.tile([48, 1], f32)
                invq = spool.tile([48, 1], f32)

                nc.vector.tensor_mul(gd[:], Gs1[:, 48:96], eye48[:])
                nc.vector.tensor_reduce(
                    ssqk[:], gd[:], axis=mybir.AxisListType.X,
                    op=mybir.AluOpType.add)
                nc.vector.tensor_mul(gd[:], Gs2[:], eye48[:])
                nc.vector.tensor_reduce(
                    ssqq[:], gd[:], axis=mybir.AxisListType.X,
                    op=mybir.AluOpType.add)
                nc.scalar.sqrt(nk[:], ssqk[:])
                nc.scalar.sqrt(nq[:], ssqq[:])
                nc.vector.tensor_scalar_max(nk[:], nk[:], EPS)
                nc.vector.tensor_scalar_max(nq[:], nq[:], EPS)
                nc.vector.reciprocal(invk[:], nk[:])
                nc.vector.reciprocal(invq[:], nq[:])

                # logits^T scaled by inv_k (rows = k-channels e)
                m1 = spool.tile([48, 48], f32)
                nc.vector.tensor_scalar(
                    m1[:], Gs1[:, 0:48], invk[:], None,
                    op0=mybir.AluOpType.mult)
                m1T = sppool.tile([48, 48], f32, tag="m1T")
                nc.tensor.transpose(m1T[:], m1[:], eye48[:])
                L = spool.tile([48, 48], f32)
                nc.vector.tensor_scalar(
                    L[:], m1T[:], invq[:], temppc[:],
                    op0=mybir.AluOpType.mult, op1=mybir.AluOpType.mult)
                nc.vector.tensor_add(L[:], L[:], maskbd[:])
                nrm = spool.tile([48, 1], f32)
                nc.vector.tensor_reduce(
                    nrm[:], L[:], axis=mybir.AxisListType.X,
                    op=mybir.AluOpType.max, negate=True)
                E = spool.tile([48, 48], f32)
                rowsum = spool.tile([48, 1], f32)
                nc.scalar.activation(
                    E[:], L[:], mybir.ActivationFunctionType.Exp,
                    bias=nrm[:], scale=1.0, accum_out=rowsum[:])
                invs = spool.tile([48, 1], f32)
                nc.vector.reciprocal(invs[:], rowsum[:])
                attn = spool.tile([48, 48], bf)
                nc.vector.tensor_scalar(
                    attn[:], E[:], invs[:], None, op0=mybir.AluOpType.mult)

                # P2^T = attn^T @ proj^T
                pt_ps = sppool.tile([48, 48], f32, tag="ptps")
                nc.tensor.matmul(pt_ps[:], attn[:], projT[:],
                                 start=True, stop=True)
                PT = spool.tile([48, 48], bf)
                nc.vector.tensor_copy(PT[:], pt_ps[:])

                # phase-2 weight stacks
                ph2A = spool.tile([128, 3 * 48], bf)
                ph2B = spool.tile([128, 3 * 48], bf)
                for dx in range(3):
                    psA = sppool.tile([128, 48], f32, tag="psA")
                    for ty in range(3):
                        nc.tensor.matmul(
                            psA[:], vA[:, (dx * 3 + ty) * 128:
                                        (dx * 3 + ty + 1) * 128],
                            PT[:], start=(ty == 0), stop=(ty == 2))
                    nc.vector.tensor_copy(ph2A[:, dx * 48:(dx + 1) * 48],
                                          psA[:])
                    psB = sppool.tile([128, 48], f32, tag="psB")
                    nc.tensor.matmul(psB[:], vB4[:, dx * 128:(dx + 1) * 128],
                                     PT[:], start=True, stop=True)
                    nc.vector.tensor_copy(ph2B[:, dx * 48:(dx + 1) * 48],
                                          psB[:])

            # ---- phase 2: final fused conv + DMA out ----
            with tc.tile_pool(name="psO", bufs=3, space="PSUM") as opool:
                for ci in range(NCH):
                    o_ps = opool.tile([48, 512], f32)
                    for dx in range(3):
                        nc.tensor.matmul(
                            o_ps[:], ph2A[:, dx * 48:(dx + 1) * 48],
                            _winA(x3A, ci, dx), start=(dx == 0), stop=False)
                        nc.tensor.matmul(
                            o_ps[:],
                            ph2B[32 * (ci // 32):32 * (ci // 32) + 16,
                                 dx * 48:(dx + 1) * 48],
                            _winB(x3B, ci, dx), start=False, stop=(dx == 2),
                            tile_position=_tp(ci))
                    o_sb = wpool.tile([48, 512], f32, tag="osb")
                    nc.scalar.copy(o_sb[:], o_ps[:])
                    nc.sync.dma_start(y[:, ci * 512:(ci + 1) * 512], o_sb[:])

    nc.compile()
    return nc


def _host_weights(qkv_w, dw_w, proj_w, temperature):
    # fused qk weights: w[(ty,c), dx*96+o] = qkv_w[o,c]*dw_w[o,0,ty,dx]
    wfull = np.einsum("oc,otd->tcdo", qkv_w[:96], dw_w[:96, 0]).astype(F32)
    wfull = wfull.reshape(144, 3, 96)          # [(ty,c), dx, o]
    wA = wfull[:128].reshape(128, 3 * 96).astype(BF16)
    wB4 = np.zeros((128, 3, 96), F32)
    for k in range(4):
        wB4[32 * k:32 * k + 16] = wfull[128:]
    wB4 = wB4.reshape(128, 3 * 96).astype(BF16)

    # v-path taps: vA[e, (dx*3+ty)*128 + r], r=(ty*48+c) slot
    vw = np.einsum("ec,etd->tdec", qkv_w[96:], dw_w[96:, 0]).astype(F32)
    # vw[ty, dx, e, c]
    vA = np.zeros((48, 9, 128), F32)
    vB4 = np.zeros((48, 3, 128), F32)
    for dx in range(3):
        for ty in range(3):
            blk = vw[ty, dx]                   # [e, c]
            for c in range(48):
                r = ty * 48 + c
                if r < 128:
                    vA[:, dx * 3 + ty, r] = blk[:, c]
                else:
                    for k in range(4):
                        vB4[:, dx, 32 * k + (r - 128)] = blk[:, c]
    vA = vA.reshape(48, 9 * 128).astype(BF16)
    vB4 = vB4.reshape(48, 3 * 128).astype(BF16)

    projT = proj_w.T.astype(BF16).copy()
    id96 = np.eye(96, dtype=F32).astype(BF16)
    eye48 = np.eye(48, dtype=F32)
    maskbd = np.full((48, 48), -1e9, F32)
    for h in range(HEADS):
        maskbd[h * HD:(h + 1) * HD, h * HD:(h + 1) * HD] = 0.0
    temppc = np.repeat(temperature.reshape(HEADS), HD).reshape(48, 1)
    temppc = temppc.astype(F32)
    return dict(wA=wA, wB4=wB4, vA=vA, vB4=vB4, projT=projT, id96=id96,
                eye48=eye48, maskbd=maskbd, temppc=temppc)


def make_in_maps(x, qkv_w, dw_w, proj_w, temperature):
    shared = _host_weights(np.asarray(qkv_w, F32), np.asarray(dw_w, F32),
                           np.asarray(proj_w, F32),
                           np.asarray(temperature, F32))
    maps = []
    for b in range(B):
        m = dict(shared)
        m["xb"] = np.ascontiguousarray(
            np.asarray(x[b], F32).reshape(C, H * W)).astype(BF16)
        maps.append(m)
    return maps


def kernel(x, qkv_w, dw_w, proj_w, temperature):
    nc = _build()
    in_maps = make_in_maps(x, qkv_w, dw_w, proj_w, temperature)
    res = bass_utils.run_bass_kernel_spmd(nc, in_maps, list(range(B)))
    out = np.stack([res.results[b]["y"].reshape(C, H, W) for b in range(B)])
    return out.astype(np.float32)


# revision 33
# speedup vs baseline: 3048.9852x; 3048.9852x over previous
"""MDTA (Restormer transposed channel attention) Trainium2 kernel.

Data-parallel over batch: 8 batch elements -> 8 NeuronCores, one each.

Per-core algorithm (matmuls take bf16 inputs, accumulate fp32 in PSUM):
  - x is uploaded as bf16 and DMA'd into a vertically-shifted padded
    "x2 stack" resident in SBUF: row (ty*48+c), ty in {0,1}, at free pos q
    holds x_pad[c, q + (ty-1)*258] over a 258-wide zero-padded image
    layout.  The third vertical tap reads the ty=0 rows at +2*258.
    Vertical conv taps thus come from partition placement / big free
    offsets, horizontal taps from small free-dim AP offsets, so the 3x3
    depthwise conv fuses into dense matmuls against the same buffer.
  - Phase 1: fused (1x1 conv + depthwise 3x3) for q,k: 6 matmuls per
    512-pixel chunk -> [96, 512] PSUM.  Chunks are transposed on the PE
    and accumulated into Gram matrices holding all per-head q.k^T blocks
    plus squared L2 norms on the diagonals.  The per-chunk post-processing
    (transpose / Gram) is emitted 1-2 chunks behind the conv so the PE
    never waits on the ACT/DVE copies.
  - Attention: logits scaled by 1/max(||q||,eps) * 1/max(||k||,eps) *
    temperature, masked block-diagonal softmax over 6-wide head blocks.
  - P2 = proj_w @ attn, then per-tap phase-2 weights C_tap^T = V_ext^T @
    P2^T are built on device (V_ext bakes the v-path 1x1 conv, depthwise
    weights and stack placement).
  - Phase 2: final output = sum over taps of C^T stacks applied to the
    same x2 stack; 6 matmuls per chunk -> [48, 512] -> DMA out.
"""

import functools
import sys

_build_conv_n = 512
_BUFS = dict(qkp=3, tp=2, wp=3, op=4)

if "/opt/trn_rl_repo" not in sys.path:
    sys.path.insert(0, "/opt/trn_rl_repo")

import ml_dtypes
import numpy as np

import concourse.bass as bass
import concourse.tile as tile
from concourse import bacc, mybir
from concourse import bass_utils

BF16 = ml_dtypes.bfloat16
F32 = np.float32

B, C, H, W = 8, 48, 256, 256
HEADS, HD = 8, 6
PW = W + 2                 # padded row width
PF = PW * (H + 2)          # padded flat image size
X2F = PF + 2 * PW + 4      # x2 buffer free size (+2 rows for the ty=2 read)
NCH = (H * W) // 512       # 128 chunks of 512 pixels (2 image rows)
EPS = 1e-12

bf = mybir.dt.bfloat16
f32 = mybir.dt.float32


def _win(t, p0, p1, ci, dx, extra=0):
    """rhs window: 512 output pixels of chunk ci at horiz tap dx."""
    off = (2 * ci + 1) * PW + dx + extra
    return t[p0:p1, off:off + 2 * PW].rearrange(
        "p (r w) -> p r w", w=PW)[:, :, 0:W]


@functools.cache
def _build(repeat=1, upto=3, xpose="dma", p1_post=True, fillsplit=8,
           fillengines=("sync",), gmode="g2"):
    # upto: 1 = x2 fill only, 2 = + phase 1, 3 = full kernel (bisect aid)
    # xpose: "pe" = TensorE transpose via PSUM; "dma" = xbar DMA transpose
    # p1_post=False: phase-1 convs only (no copy/transpose/Gram) - bisect aid
    # fillsplit/fillengines: x2 interior DMA chunking and issuing engines
    conv_n = _build_conv_n  # timing aid: shrink conv rhs N (breaks output)
    nc = bacc.Bacc("TRN2", target_bir_lowering=False, debug=False)

    # xb rows are host-padded to 258 ([0, row, 0]) so the x2 interior fill
    # is a fully contiguous DMA and the pad columns need no memset.
    xb = nc.dram_tensor("xb", [C, H * PW], bf, kind="ExternalInput").ap()
    w01_d = nc.dram_tensor("w01", [96, 3 * 96], bf, kind="ExternalInput").ap()
    w2_d = nc.dram_tensor("w2", [48, 3 * 96], bf, kind="ExternalInput").ap()
    vA_d = nc.dram_tensor("vA", [48, 6 * 96], bf, kind="ExternalInput").ap()
    vB_d = nc.dram_tensor("vB", [48, 3 * 48], bf, kind="ExternalInput").ap()
    projT_d = nc.dram_tensor("projT", [48, 48], bf, kind="ExternalInput").ap()
    id96_d = nc.dram_tensor("id96", [96, 96], bf, kind="ExternalInput").ap()
    eye48_d = nc.dram_tensor("eye48", [48, 48], f32, kind="ExternalInput").ap()
    mask_d = nc.dram_tensor("maskbd", [48, 48], f32, kind="ExternalInput").ap()
    temp_d = nc.dram_tensor("temppc", [48, 1], f32, kind="ExternalInput").ap()
    y = nc.dram_tensor("y", [C, H * W], f32, kind="ExternalOutput").ap()

    with tile.TileContext(nc) as tc:
        with (
            tc.tile_pool(name="const", bufs=1) as cpool,
            tc.tile_pool(name="x2", bufs=1) as x2pool,
            tc.tile_pool(name="work", bufs=_BUFS["wp"]) as wpool,
            tc.tile_pool(name="small", bufs=1) as spool,
        ):
            # ---- constants to SBUF ----
            w01 = cpool.tile([96, 3 * 96], bf)
            w2 = cpool.tile([48, 3 * 96], bf)
            vA = cpool.tile([48, 6 * 96], bf)
            vB = cpool.tile([48, 3 * 48], bf)
            projT = cpool.tile([48, 48], bf)
            id96 = cpool.tile([96, 96], bf)
            eye48 = cpool.tile([48, 48], f32)
            maskbd = cpool.tile([48, 48], f32)
            temppc = cpool.tile([48, 1], f32)
            for dst, src in [(w01, w01_d), (w2, w2_d), (vA, vA_d),
                             (vB, vB_d), (projT, projT_d), (id96, id96_d),
                             (eye48, eye48_d), (maskbd, mask_d),
                             (temppc, temp_d)]:
                nc.sync.dma_start(dst[:], src[:])

            x2 = x2pool.tile([96, X2F], bf)

            for _rep in range(repeat):
                # ---- x2 stack: zero borders, DMA padded interiors ----
                # ty block tb at partitions 48*tb holds x_pad rows starting
                # at q-row (2-tb); rows are host-padded so the interior copy
                # is contiguous.  Top/bottom pad rows are memset (memsets
                # conservatively cover both blocks; DMAs overwrite after).
                nc.vector.memset(x2[0:96, 0:2 * PW], 0.0)        # top rows
                nc.vector.memset(x2[0:96, (H + 1) * PW:X2F], 0.0)  # bottom
                part = (H // fillsplit) * PW
                engs = [getattr(nc, e) for e in fillengines]
                di = 0
                for tb in range(2):
                    o0 = (2 - tb) * PW
                    for hh in range(fillsplit):
                        engs[di % len(engs)].dma_start(
                            x2[48 * tb:48 * tb + 48,
                               o0 + hh * part:o0 + (hh + 1) * part],
                            xb[:, hh * part:(hh + 1) * part])
                        di += 1

                if upto < 2:
                    continue

                # ---- phase 1: qk fused conv + Gram (SW-pipelined) ----
                with (
                    tc.tile_pool(name="psG", bufs=1, space="PSUM") as gpool,
                    tc.tile_pool(name="psqk", bufs=_BUFS["qkp"], space="PSUM") as qkp,
                    tc.tile_pool(name="pst", bufs=_BUFS["tp"], space="PSUM") as tpool,
                ):
                    gw = 96 if gmode == "g2" else 48
                    G1 = gpool.tile([48, gw], f32)  # k-rows: [k.q | k.k]
                    G2 = (gpool.tile([48, 48], f32, name="G2", tag="G2")
                          if gmode == "g2" else None)  # q.q
                    ssqacc = spool.tile([96, NCH], f32)
                    qk_sbs, qkTs = {}, {}

                    def conv_qk(ci):
                        qk_ps = qkp.tile([96, 512], f32)
                        for dx in range(3):
                            nc.tensor.matmul(
                                qk_ps[:, 0:conv_n], w01[:, dx * 96:(dx + 1) * 96],
                                _win(x2, 0, 96, ci, dx)[:, 0:1, 0:conv_n]
                                if conv_n < 512 else _win(x2, 0, 96, ci, dx),
                                start=(dx == 0), stop=False)
                            nc.tensor.matmul(
                                qk_ps[:, 0:conv_n], w2[:, dx * 96:(dx + 1) * 96],
                                _win(x2, 0, 48, ci, dx, extra=2 * PW)[:, 0:1, 0:conv_n]
                                if conv_n < 512 else
                                _win(x2, 0, 48, ci, dx, extra=2 * PW),
                                start=False, stop=(dx == 2))
                        if not p1_post:
                            return
                        qk_sb = wpool.tile([96, 512], bf, tag="qksb")
                        nc.scalar.copy(qk_sb[:], qk_ps[:])
                        if gmode == "sq":
                            sq_sb = wpool.tile([96, 512], bf, tag="sqsb")
                            nc.scalar.activation(
                                sq_sb[:], qk_ps[:],
                                mybir.ActivationFunctionType.Square,
                                accum_out=ssqacc[:, ci:ci + 1])
                        qk_sbs[ci] = qk_sb

                    def transp(ci):
                        qk_sb = qk_sbs.pop(ci)
                        qkT = wpool.tile([128, 384], bf, tag="qkT")
                        if xpose == "dma":
                            for j in range(4):
                                nc.sync.dma_start_transpose(
                                    qkT[:, j * 96:(j + 1) * 96],
                                    qk_sb[:, j * 128:(j + 1) * 128])
                        else:
                            ps_t = tpool.tile([128, 384], bf)
                            for j in range(4):
                                nc.tensor.transpose(
                                    ps_t[:, j * 96:(j + 1) * 96],
                                    qk_sb[:, j * 128:(j + 1) * 128], id96[:])
                            nc.vector.tensor_copy(qkT[:], ps_t[:])
                        qkTs[ci] = qkT

                    def gram(ci):
                        qkT = qkTs.pop(ci)
                        for j in range(4):
                            first = (ci == 0 and j == 0)
                            last = (ci == NCH - 1 and j == 3)
                            nc.tensor.matmul(
                                G1[:], qkT[:, j * 96 + 48:j * 96 + 96],
                                qkT[:, j * 96:j * 96 + gw],
                                start=first, stop=last)
                            if gmode == "g2":
                                nc.tensor.matmul(
                                    G2[:], qkT[:, j * 96:j * 96 + 48],
                                    qkT[:, j * 96:j * 96 + 48],
                                    start=first, stop=last)

                    if p1_post:
                        for ci in range(NCH):
                            conv_qk(ci)
                            if ci >= 1:
                                transp(ci - 1)
                            if ci >= 2:
                                gram(ci - 2)
                        transp(NCH - 1)
                        gram(NCH - 2)
                        gram(NCH - 1)
                    else:
                        for ci in range(NCH):
                            conv_qk(ci)
                        nc.tensor.matmul(
                            G1[:], w01[:, 0:48],
                            _win(x2, 0, 96, 0, 0)[:, 0:1, 0:48],
                            start=True, stop=True)

                    Gs1 = spool.tile([48, 48], f32)
                    nc.vector.tensor_copy(Gs1[:], G1[:, 0:48])
                    ssqk2 = spool.tile([48, 1], f32)
                    ssqq2 = spool.tile([48, 1], f32)
                    if gmode == "g2":
                        gd = spool.tile([48, 48], f32)
                        nc.vector.tensor_mul(gd[:], G1[:, 48:96], eye48[:])
                        nc.vector.tensor_reduce(
                            ssqk2[:], gd[:], axis=mybir.AxisListType.X,
                            op=mybir.AluOpType.add)
                        gd2 = spool.tile([48, 48], f32)
                        nc.vector.tensor_mul(gd2[:], G2[:], eye48[:])
                        nc.vector.tensor_reduce(
                            ssqq2[:], gd2[:], axis=mybir.AxisListType.X,
                            op=mybir.AluOpType.add)

                if upto < 3:
                    continue

                # ---- attention (tiny, partitions 0-47) ----
                with tc.tile_pool(name="psS", bufs=1, space="PSUM") as spp:
                    nk = spool.tile([48, 1], f32)
                    nq = spool.tile([48, 1], f32)
                    invk = spool.tile([48, 1], f32)
                    invq = spool.tile([48, 1], f32)
                    if gmode == "sq":
                        ssqf = spool.tile([96, 1], f32)
                        nc.vector.tensor_reduce(
                            ssqf[:], ssqacc[:], axis=mybir.AxisListType.X,
                            op=mybir.AluOpType.add)
                        ssqk = spool.tile([48, 1], f32)
                        nc.sync.dma_start(ssqk[:], ssqf[48:96, :])
                        nc.scalar.sqrt(nk[:], ssqk[:])
                        nc.scalar.sqrt(nq[:], ssqf[0:48, :])
                    else:
                        nc.scalar.sqrt(nk[:], ssqk2[:])
                        nc.scalar.sqrt(nq[:], ssqq2[:])
                    nc.vector.tensor_scalar_max(nk[:], nk[:], EPS)
                    nc.vector.tensor_scalar_max(nq[:], nq[:], EPS)
                    nc.vector.reciprocal(invk[:], nk[:])
                    nc.vector.reciprocal(invq[:], nq[:])

                    # logits^T scaled by inv_k (rows = k-channels e)
                    m1 = spool.tile([48, 48], f32)
                    nc.vector.tensor_scalar(
                        m1[:], Gs1[:], invk[:], None,
                        op0=mybir.AluOpType.mult)
                    m1T = spp.tile([48, 48], f32, tag="m1T")
                    nc.tensor.transpose(m1T[:], m1[:], eye48[:])
                    L = spool.tile([48, 48], f32)
                    nc.vector.tensor_scalar(
                        L[:], m1T[:], invq[:], temppc[:],
                        op0=mybir.AluOpType.mult, op1=mybir.AluOpType.mult)
                    nc.vector.tensor_add(L[:], L[:], maskbd[:])
                    nrm = spool.tile([48, 1], f32)
                    nc.vector.tensor_reduce(
                        nrm[:], L[:], axis=mybir.AxisListType.X,
                        op=mybir.AluOpType.max, negate=True)
                    E = spool.tile([48, 48], f32)
                    rowsum = spool.tile([48, 1], f32)
                    nc.scalar.activation(
                        E[:], L[:], mybir.ActivationFunctionType.Exp,
                        bias=nrm[:], scale=1.0, accum_out=rowsum[:])
                    invs = spool.tile([48, 1], f32)
                    nc.vector.reciprocal(invs[:], rowsum[:])
                    attn = spool.tile([48, 48], bf)
                    nc.vector.tensor_scalar(
                        attn[:], E[:], invs[:], None,
                        op0=mybir.AluOpType.mult)

                    # P2^T = attn^T @ proj^T
                    pt_ps = spp.tile([48, 48], f32, tag="ptps")
                    nc.tensor.matmul(pt_ps[:], attn[:], projT[:],
                                     start=True, stop=True)
                    PT = spool.tile([48, 48], bf)
                    nc.vector.tensor_copy(PT[:], pt_ps[:])

                    # phase-2 weight stacks
                    ph2a = spool.tile([96, 3 * 48], bf)
                    ph2b = spool.tile([48, 3 * 48], bf)
                    for dx in range(3):
                        psA = spp.tile([96, 48], f32, tag="psA")
                        for ty in range(2):
                            nc.tensor.matmul(
                                psA[:], vA[:, (dx * 2 + ty) * 96:
                                            (dx * 2 + ty + 1) * 96],
                                PT[:], start=(ty == 0), stop=(ty == 1))
                        nc.vector.tensor_copy(
                            ph2a[:, dx * 48:(dx + 1) * 48], psA[:])
                        psB = spp.tile([48, 48], f32, tag="psB")
                        nc.tensor.matmul(psB[:],
                                         vB[:, dx * 48:(dx + 1) * 48],
                                         PT[:], start=True, stop=True)
                        nc.vector.tensor_copy(
                            ph2b[:, dx * 48:(dx + 1) * 48], psB[:])

                # ---- phase 2: final fused conv + DMA out ----
                with tc.tile_pool(name="psO", bufs=_BUFS["op"], space="PSUM") as opool:
                    for ci in range(NCH):
                        o_ps = opool.tile([48, 512], f32)
                        for dx in range(3):
                            nc.tensor.matmul(
                                o_ps[:], ph2a[:, dx * 48:(dx + 1) * 48],
                                _win(x2, 0, 96, ci, dx),
                                start=(dx == 0), stop=False)
                            nc.tensor.matmul(
                                o_ps[:], ph2b[:, dx * 48:(dx + 1) * 48],
                                _win(x2, 0, 48, ci, dx, extra=2 * PW),
                                start=False, stop=(dx == 2))
                        if ci % 2 == 0:
                            o_sb = wpool.tile([48, 1024], f32, tag="osb")
                        nc.scalar.copy(o_sb[:, (ci % 2) * 512:
                                            (ci % 2 + 1) * 512], o_ps[:])
                        if ci % 2 == 1:
                            nc.sync.dma_start(
                                y[:, (ci - 1) * 512:(ci + 1) * 512], o_sb[:])

    nc.compile()
    return nc


def _host_weights(qkv_w, dw_w, proj_w, temperature):
    # fused qk weights: w[(ty,c), dx*96+o] = qkv_w[o,c]*dw_w[o,0,ty,dx]
    wfull = np.einsum("oc,otd->tcdo", qkv_w[:96], dw_w[:96, 0]).astype(F32)
    wfull = wfull.reshape(144, 3, 96)          # [(ty,c), dx, o]
    w01 = wfull[:96].reshape(96, 3 * 96).astype(BF16)
    w2 = wfull[96:].reshape(48, 3 * 96).astype(BF16)

    # v-path taps: vA[e, (dx*2+ty)*96 + r] (ty in {0,1}), vB[e, dx*48+c]
    vw = np.einsum("ec,etd->tdec", qkv_w[96:], dw_w[96:, 0]).astype(F32)
    vA = np.zeros((48, 6, 96), F32)
    vB = np.zeros((48, 3, 48), F32)
    for dx in range(3):
        for ty in range(2):
            for c in range(48):
                vA[:, dx * 2 + ty, ty * 48 + c] = vw[ty, dx, :, c]
        vB[:, dx, :] = vw[2, dx]               # [e, c]
    vA = vA.reshape(48, 6 * 96).astype(BF16)
    vB = vB.reshape(48, 3 * 48).astype(BF16)

    projT = proj_w.T.astype(BF16).copy()
    id96 = np.eye(96, dtype=F32).astype(BF16)
    eye48 = np.eye(48, dtype=F32)
    maskbd = np.full((48, 48), -1e9, F32)
    for h in range(HEADS):
        maskbd[h * HD:(h + 1) * HD, h * HD:(h + 1) * HD] = 0.0
    temppc = np.repeat(temperature.reshape(HEADS), HD).reshape(48, 1)
    temppc = temppc.astype(F32)
    return dict(w01=w01, w2=w2, vA=vA, vB=vB, projT=projT, id96=id96,
                eye48=eye48, maskbd=maskbd, temppc=temppc)


def make_in_maps(x, qkv_w, dw_w, proj_w, temperature):
    shared = _host_weights(np.asarray(qkv_w, F32), np.asarray(dw_w, F32),
                           np.asarray(proj_w, F32),
                           np.asarray(temperature, F32))
    xp = np.zeros((B, C, H, PW), F32)
    xp[:, :, :, 1:1 + W] = np.asarray(x, F32).reshape(B, C, H, W)
    xp = xp.reshape(B, C, H * PW).astype(BF16)
    maps = []
    for b in range(B):
        m = dict(shared)
        m["xb"] = xp[b]
        maps.append(m)
    return maps


def kernel(x, qkv_w, dw_w, proj_w, temperature):
    nc = _build()
    in_maps = make_in_maps(x, qkv_w, dw_w, proj_w, temperature)
    res = bass_utils.run_bass_kernel_spmd(nc, in_maps, list(range(B)))
    out = np.stack([res.results[b]["y"].reshape(C, H, W) for b in range(B)])
    return out.astype(np.float32)
